# revision 3
# baseline (speedup 1.0000x reference)
"""Trainium2 Bass kernel for the 2-layer GRU + BN + maxpool + FC model (v2).

Time-sharded across 8 cores x 4 windows/core (32 windows of 64 output
steps, 16-step warmup exploiting the GRU's contractive state). Each core
interleaves its 4 windows as 2 fused pairs: all elementwise ops are
[128, 2*192]-wide over a window pair, halving fixed per-instruction
costs, while the two pairs provide ILP to cover each other's serial
recurrence latency.

Per step the input projections (x->gx1, h1->gx2) are injected directly
into the gate PSUM banks by the tensor engine (no SBUF staging of gx, no
PSUM->SBUF copies), accumulated together with the recurrent W_hh
matmuls. Layer 2 lags layer 1 by one step. Everything is bf16 (PSUM
accumulation in fp32): matmuls run at 1 cycle/row instead of fp32's 4.

Max-pooling over time happens per-step on the DVE; warmup steps are
suppressed with a -BIG additive mask supplied as data so the program
stays SPMD-uniform across cores. BatchNorm1 is folded into the L2 input
projection; BN2 + tanh + FC run on the host on the tiny pooled result.
"""

from contextlib import ExitStack

import numpy as np
import ml_dtypes

import concourse.bass as bass
import concourse.bacc as bacc
import concourse.tile as tile
from concourse import mybir
from concourse.bass_utils import run_bass_kernel_spmd

# Model dims (hardcoded per spec)
B, T, D, H1, H2, O = 64, 2048, 64, 256, 128, 10
EPS = 1e-5

NCORES = 8
KW = 4                      # windows per core
NPAIR = 2                   # window pairs per core
WINLEN = T // (NCORES * KW)  # 64 output steps per window
WU = 16                     # warmup steps
TW = WINLEN + WU            # 80 L1 steps per window
NSTEP = TW + 1              # 81 bank-steps (L2 lags L1 by one step)
C = 8                       # x-chunk steps
NCH = TW // C               # 10 chunks

BIG = 60000.0

DT = mybir.dt.bfloat16
NPD = ml_dtypes.bfloat16
F32 = mybir.dt.float32

# Gate m-tile column offsets inside wa/whh1 (torch gate order r,z,n)
# wa/whh1 columns: [r0 r1 z0 z1 n0 n1] * 128
# wb/whh2 columns: [r2 z2 n2] * 128


def build_bass():
    nc = bacc.Bacc("TRN2", target_bir_lowering=False, debug=False,
                   num_devices=NCORES)

    # ---- DRAM I/O (per-core data; program identical across cores) ----
    xts = [nc.dram_tensor(f"xt{p}", [D + 1, TW * 2 * B], DT,
                          kind="ExternalInput").ap() for p in range(NPAIR)]
    wa_d = nc.dram_tensor("wa", [D + 1, 6 * 128], DT, kind="ExternalInput").ap()
    whh1_d = nc.dram_tensor("whh1", [H1, 6 * 128], DT, kind="ExternalInput").ap()
    wb_d = nc.dram_tensor("wb", [H1, 3 * 128], DT, kind="ExternalInput").ap()
    whh2_d = nc.dram_tensor("whh2", [H2, 3 * 128], DT, kind="ExternalInput").ap()
    b2rz_d = nc.dram_tensor("b2rz", [1, 256], DT, kind="ExternalInput").ap()
    bhn1_d = nc.dram_tensor("bhn1", [1, 256], DT, kind="ExternalInput").ap()
    bhn2_d = nc.dram_tensor("bhn2", [1, 128], DT, kind="ExternalInput").ap()
    b2n_d = nc.dram_tensor("b2n", [1, 128], DT, kind="ExternalInput").ap()
    idn_d = nc.dram_tensor("idn", [128, 128], DT, kind="ExternalInput").ap()
    maskw_d = nc.dram_tensor("maskw", [128, WU * KW], DT,
                             kind="ExternalInput").ap()
    pmax_d = nc.dram_tensor("pmax", [128, NPAIR * 128], DT,
                            kind="ExternalOutput").ap()

    with tile.TileContext(nc) as tc, ExitStack() as ctx:
        singles = ctx.enter_context(tc.tile_pool(name="singles", bufs=1))
        xcp = ctx.enter_context(tc.tile_pool(name="xc", bufs=3))
        hbp = ctx.enter_context(tc.tile_pool(name="hb", bufs=2))
        work = ctx.enter_context(tc.tile_pool(name="work", bufs=2))
        # PSUM: per pair, two 2-bank tiles: grz = [Pr | Pz], gng = [Pn | Pgn]
        grzp = [ctx.enter_context(tc.tile_pool(name=f"grz{p}", bufs=1, space="PSUM"))
                for p in range(NPAIR)]
        gngp = [ctx.enter_context(tc.tile_pool(name=f"gng{p}", bufs=1, space="PSUM"))
                for p in range(NPAIR)]

        # ---- constants into SBUF ----
        wa_sb = singles.tile([D + 1, 768], DT)
        nc.sync.dma_start(wa_sb[:], wa_d[:])
        whh1_sb = singles.tile([128, 2 * 768], DT)   # k0 | k1
        nc.sync.dma_start(whh1_sb[:, 0:768], whh1_d[0:128, :])
        nc.sync.dma_start(whh1_sb[:, 768:1536], whh1_d[128:256, :])
        wb_sb = singles.tile([128, 2 * 384], DT)     # k0 | k1
        nc.sync.dma_start(wb_sb[:, 0:384], wb_d[0:128, :])
        nc.sync.dma_start(wb_sb[:, 384:768], wb_d[128:256, :])
        whh2_sb = singles.tile([H2, 384], DT)
        nc.sync.dma_start(whh2_sb[:], whh2_d[:])
        b2rz_sb = singles.tile([1, 256], DT)
        nc.sync.dma_start(b2rz_sb[:], b2rz_d[:])
        bhn1_sb = singles.tile([1, 256], DT)
        nc.sync.dma_start(bhn1_sb[:], bhn1_d[:])
        bhn2_sb = singles.tile([1, 128], DT)
        nc.sync.dma_start(bhn2_sb[:], bhn2_d[:])
        b2n_sb = singles.tile([1, 128], DT)
        nc.sync.dma_start(b2n_sb[:], b2n_d[:])
        idn_sb = singles.tile([128, 128], DT)
        nc.sync.dma_start(idn_sb[:], idn_d[:])
        maskw_sb = singles.tile([128, WU * KW], DT)
        nc.sync.dma_start(maskw_sb[:], maskw_d[:])
        ones_sb = singles.tile([1, 128], DT)
        nc.vector.memset(ones_sb[:], 1.0)
        pmax_sb = [singles.tile([128, 128], DT, tag=f"pm{p}", name=f"pm{p}")
                   for p in range(NPAIR)]
        for p in range(NPAIR):
            nc.vector.memset(pmax_sb[p][:], -BIG)

        # ---- prologue: first x chunks, zero h ----
        xcs = [[None] * NCH for _ in range(NPAIR)]
        for p in range(NPAIR):
            for k in (0, 1):
                xcs[p][k] = xcp.tile([D + 1, C * 128], DT, tag=f"xc{p}", name=f"xc{p}")
                nc.sync.dma_start(xcs[p][k][:], xts[p][:, k * C * 128:(k + 1) * C * 128])
        h_cur = [None] * NPAIR
        for p in range(NPAIR):
            h = hbp.tile([128, 384], DT, tag=f"hb{p}", name=f"hb{p}")
            nc.vector.memset(h[:], 0.0)
            h_cur[p] = h

        # Exactly ONE start=True matmul per PSUM bank per step: a start=True
        # clears the has-written bits of the WHOLE bank, so any section whose
        # group opened earlier would silently lose its accumulation. All other
        # matmuls use start=False (first write to an element lazily
        # overwrites, later writes accumulate).
        opened = {}

        def mm(key, dst, lhsT, rhs, stop):
            st = not opened.get(key, False)
            opened[key] = True
            nc.tensor.matmul(dst, lhsT, rhs, start=st, stop=stop)

        def emit_early_rz(s):
            """Allocate Pr/Pz banks for step s; emit their h-independent
            matmuls (gx1 r/z from x, L2 bias rows)."""
            k, sl = s // C, s % C
            l1, l2 = s < TW, s >= 1
            grz = [None] * NPAIR
            for p in range(NPAIR):
                grz[p] = grzp[p].tile([128, 2, 512], F32, tag=f"grz{p}", name=f"grz{p}")
            for p in range(NPAIR):
                Pr, Pz = grz[p][:, 0, :], grz[p][:, 1, :]
                if l1:
                    xs = xcs[p][k][:, sl * 128:(sl + 1) * 128]
                    for (P, g) in ((Pr, 'r'), (Pz, 'z')):
                        gi = 0 if g == 'r' else 1
                        for m in range(2):
                            mm((s, p, g), P[:, m * 128:(m + 1) * 128],
                               wa_sb[:, (2 * gi + m) * 128:(2 * gi + m + 1) * 128],
                               xs, stop=False)
                if l2:
                    mm((s, p, 'r'), Pr[:, 256:384], b2rz_sb[:, 0:128],
                       ones_sb[:], stop=False)
                    mm((s, p, 'z'), Pz[:, 256:384], b2rz_sb[:, 128:256],
                       ones_sb[:], stop=False)
            return grz

        def emit_early_ng(s):
            """Allocate Pn/Pgn banks for step s; emit their h-independent
            matmuls (gx1 n from x, bhn/b2n bias rows)."""
            k, sl = s // C, s % C
            l1, l2 = s < TW, s >= 1
            gng = [None] * NPAIR
            for p in range(NPAIR):
                gng[p] = gngp[p].tile([128, 2, 512], F32, tag=f"gng{p}", name=f"gng{p}")
            for p in range(NPAIR):
                Pn, Pg = gng[p][:, 0, :], gng[p][:, 1, :]
                if l1:
                    xs = xcs[p][k][:, sl * 128:(sl + 1) * 128]
                    # gx1 n into Pgn m0/m1 (closed later by the idn tn-add)
                    for m in range(2):
                        mm((s, p, 'g'), Pg[:, m * 128:(m + 1) * 128],
                           wa_sb[:, (4 + m) * 128:(5 + m) * 128], xs, stop=False)
                    # bhn1 into Pn m0/m1
                    for m in range(2):
                        mm((s, p, 'n'), Pn[:, m * 128:(m + 1) * 128],
                           bhn1_sb[:, m * 128:(m + 1) * 128], ones_sb[:], stop=False)
                if l2:
                    mm((s, p, 'n'), Pn[:, 256:384], bhn2_sb[:], ones_sb[:], stop=False)
                    mm((s, p, 'g'), Pg[:, 256:384], b2n_sb[:], ones_sb[:], stop=False)
            return gng

        # ---- main loop over bank-steps (early matmuls pipelined one ahead) ----
        grz_cur = emit_early_rz(0)
        gng_cur = emit_early_ng(0)
        for s in range(NSTEP):
            k = s // C
            sl = s % C
            # chunk prefetch (2 ahead)
            if sl == 0 and k + 2 < NCH:
                for p in range(NPAIR):
                    xcs[p][k + 2] = xcp.tile([D + 1, C * 128], DT, tag=f"xc{p}",
                                             name=f"xc{p}")
                    nc.sync.dma_start(
                        xcs[p][k + 2][:],
                        xts[p][:, (k + 2) * C * 128:(k + 3) * C * 128])

            l1 = s < TW        # layer-1 active this bank-step
            l2 = s >= 1        # layer-2 active (computes h2(s-1))
            grz, gng = grz_cur, gng_cur

            # ---- late matmuls: contract h(s-1) ----
            for p in range(NPAIR):
                Pr, Pz = grz[p][:, 0, :], grz[p][:, 1, :]
                Pn, Pg = gng[p][:, 0, :], gng[p][:, 1, :]
                h = h_cur[p]
                hk = [h[:, 0:128], h[:, 128:256]]   # h1 k-halves
                h2s = h[:, 256:384]
                # close Pr fully first (sigmoid_r is on the critical path),
                # then Pz, then the n banks
                for (P, g) in ((Pr, 'r'), (Pz, 'z')):
                    gi = 0 if g == 'r' else 1
                    if l1:
                        for m in range(2):
                            col = (2 * gi + m) * 128
                            mm((s, p, g), P[:, m * 128:(m + 1) * 128],
                               whh1_sb[:, col:col + 128], hk[0], stop=False)
                            mm((s, p, g), P[:, m * 128:(m + 1) * 128],
                               whh1_sb[:, 768 + col:768 + col + 128], hk[1],
                               stop=True)
                    if l2:
                        mm((s, p, g), P[:, 256:384],
                           wb_sb[:, gi * 128:(gi + 1) * 128], hk[0], stop=False)
                        mm((s, p, g), P[:, 256:384],
                           wb_sb[:, 384 + gi * 128:384 + (gi + 1) * 128], hk[1],
                           stop=False)
                        mm((s, p, g), P[:, 256:384],
                           whh2_sb[:, gi * 128:(gi + 1) * 128], h2s, stop=True)
                if l1:
                    # W_hh1 for n (close Pn L1 sections)
                    for m in range(2):
                        col = (4 + m) * 128
                        mm((s, p, 'n'), Pn[:, m * 128:(m + 1) * 128],
                           whh1_sb[:, col:col + 128], hk[0], stop=False)
                        mm((s, p, 'n'), Pn[:, m * 128:(m + 1) * 128],
                           whh1_sb[:, 768 + col:768 + col + 128], hk[1],
                           stop=True)
                if l2:
                    # L2 n: whh2 into Pn, gx2 into Pgn (close L2 sections)
                    mm((s, p, 'n'), Pn[:, 256:384], whh2_sb[:, 256:384],
                       h2s, stop=True)
                    mm((s, p, 'g'), Pg[:, 256:384], wb_sb[:, 256:384],
                       hk[0], stop=False)
                    mm((s, p, 'g'), Pg[:, 256:384], wb_sb[:, 640:768],
                       hk[1], stop=False)

            # ---- elementwise (pair-fused) ----
            srz = [work.tile([128, 768], DT, tag=f"srz{p}", name=f"srz{p}")
                   for p in range(NPAIR)]
            zc = [work.tile([128, 384], DT, tag=f"zc{p}", name=f"zc{p}")
                  for p in range(NPAIR)]
            tn = [work.tile([128, 384], DT, tag=f"tn{p}", name=f"tn{p}")
                  for p in range(NPAIR)]
            ntl = [work.tile([128, 384], DT, tag=f"ntl{p}", name=f"ntl{p}")
                   for p in range(NPAIR)]
            wzh = [work.tile([128, 384], DT, tag=f"wzh{p}", name=f"wzh{p}")
                   for p in range(NPAIR)]
            u = [work.tile([128, 384], DT, tag=f"u{p}", name=f"u{p}")
                 for p in range(NPAIR)]
            h_new = [hbp.tile([128, 384], DT, tag=f"hb{p}", name=f"hb{p}")
                     for p in range(NPAIR)]

            for p in range(NPAIR):
                nc.scalar.activation(srz[p][:, 0:384], grz[p][:, 0, 0:384],
                                     mybir.ActivationFunctionType.Sigmoid)
            for p in range(NPAIR):
                nc.scalar.activation(srz[p][:, 384:768], grz[p][:, 1, 0:384],
                                     mybir.ActivationFunctionType.Sigmoid)
            for p in range(NPAIR):
                nc.vector.tensor_mul(tn[p][:], srz[p][:, 0:384], gng[p][:, 0, 0:384])
            for p in range(NPAIR):
                # Pgn += tn on the tensor engine (closes the Pgn sections)
                mm((s, p, 'g'), gng[p][:, 1, 0:384], idn_sb[:], tn[p][:],
                   stop=True)
            for p in range(NPAIR):
                # zc = 1 - z (off critical path, right after sigmoid_z)
                nc.vector.tensor_scalar(zc[p][:], srz[p][:, 384:768], -1.0, 1.0,
                                        op0=mybir.AluOpType.mult,
                                        op1=mybir.AluOpType.add)
            for p in range(NPAIR):
                nc.gpsimd.tensor_mul(wzh[p][:], srz[p][:, 384:768], h_cur[p][:])
            for p in range(NPAIR):
                nc.scalar.activation(ntl[p][:], gng[p][:, 1, 0:384],
                                     mybir.ActivationFunctionType.Tanh)
            # h' = z*h + (1-z)*n ; restrict written sections at the edges
            if s == 0:
                lo, hi = 0, 256
            elif s == NSTEP - 1:
                lo, hi = 256, 384
            else:
                lo, hi = 0, 384
            for p in range(NPAIR):
                nc.vector.tensor_mul(u[p][:, lo:hi], zc[p][:, lo:hi],
                                     ntl[p][:, lo:hi])
            for p in range(NPAIR):
                nc.vector.tensor_add(h_new[p][:, lo:hi], wzh[p][:, lo:hi],
                                     u[p][:, lo:hi])
            if s == 0:
                for p in range(NPAIR):
                    nc.vector.memset(h_new[p][:, 256:384], 0.0)

            # ---- pooling of h2(s-1) ----
            if 1 <= s <= WU:
                for p in range(NPAIR):
                    for w in range(2):
                        nc.vector.scalar_tensor_tensor(
                            pmax_sb[p][:, w * 64:(w + 1) * 64],
                            h_new[p][:, 256 + w * 64:256 + (w + 1) * 64],
                            maskw_sb[:, (s - 1) * KW + 2 * p + w:
                                     (s - 1) * KW + 2 * p + w + 1],
                            pmax_sb[p][:, w * 64:(w + 1) * 64],
                            op0=mybir.AluOpType.add, op1=mybir.AluOpType.max)
            elif s > WU:
                for p in range(NPAIR):
                    nc.vector.tensor_max(pmax_sb[p][:], pmax_sb[p][:],
                                         h_new[p][:, 256:384])

            for p in range(NPAIR):
                h_cur[p] = h_new[p]
            if s + 1 < NSTEP:
                grz_cur = emit_early_rz(s + 1)
                gng_cur = emit_early_ng(s + 1)

        # ---- epilogue ----
        for p in range(NPAIR):
            nc.sync.dma_start(pmax_d[:, p * 128:(p + 1) * 128], pmax_sb[p][:])

    nc.compile()
    return nc


def prep_core_inputs(inputs):
    """Host-side data prep: per-core input dicts (layout/slice/cast only)."""
    x = np.asarray(inputs['x'], np.float32)
    W_ih1 = np.asarray(inputs['W_ih1'], np.float32)
    W_hh1 = np.asarray(inputs['W_hh1'], np.float32)
    b_ih1 = np.asarray(inputs['b_ih1'], np.float32)
    b_hh1 = np.asarray(inputs['b_hh1'], np.float32)
    W_ih2 = np.asarray(inputs['W_ih2'], np.float32)
    W_hh2 = np.asarray(inputs['W_hh2'], np.float32)
    b_ih2 = np.asarray(inputs['b_ih2'], np.float32)
    b_hh2 = np.asarray(inputs['b_hh2'], np.float32)
    g1 = np.asarray(inputs['bn1_gamma'], np.float32)
    be1 = np.asarray(inputs['bn1_beta'], np.float32)
    m1 = np.asarray(inputs['bn1_mean'], np.float32)
    v1 = np.asarray(inputs['bn1_var'], np.float32)

    s1 = g1 / np.sqrt(v1 + EPS)
    W2p = W_ih2 * s1[None, :]                      # BN1 folded
    b2extra = W_ih2 @ (be1 - m1 * s1)
    b2row = (b2extra + b_ih2 +
             np.concatenate([b_hh2[0:H2], b_hh2[H2:2 * H2], np.zeros(H2, np.float32)]))
    wa = np.vstack([
        W_ih1.T,
        (b_ih1 + np.concatenate([b_hh1[0:H1], b_hh1[H1:2 * H1],
                                 np.zeros(H1, np.float32)]))[None, :],
    ])  # [65, 768]

    base = dict(
        wa=wa.astype(NPD),
        whh1=W_hh1.T.astype(NPD).copy(),
        wb=W2p.T.astype(NPD).copy(),
        whh2=W_hh2.T.astype(NPD).copy(),
        b2rz=b2row[None, 0:256].astype(NPD).copy(),
        bhn1=b_hh1[None, 2 * H1:3 * H1].astype(NPD).copy(),
        bhn2=b_hh2[None, 2 * H2:3 * H2].astype(NPD).copy(),
        b2n=b2row[None, 256:384].astype(NPD).copy(),
        idn=np.eye(128, dtype=np.float32).astype(NPD),
    )

    in_maps = []
    for core in range(NCORES):
        m = dict(base)
        maskw = np.full((128, WU * KW), -BIG, np.float32)
        for p in range(NPAIR):
            cols = []
            for j in range(2):
                widx = core * KW + 2 * p + j
                t0 = 0 if widx == 0 else widx * WINLEN - WU
                if widx == 0:
                    maskw[:, np.arange(WU) * KW + 2 * p + j] = 0.0
                xw = x[:, t0:t0 + TW, :]                   # [B, TW, D]
                cols.append(np.transpose(xw, (2, 1, 0)))   # [D, TW, B]
            xp = np.concatenate(cols, axis=2)              # [D, TW, 2B]
            xp = np.concatenate([xp, np.ones((1, TW, 2 * B), np.float32)], axis=0)
            m[f"xt{p}"] = np.ascontiguousarray(xp.reshape(D + 1, TW * 2 * B)).astype(NPD)
        m["maskw"] = maskw.astype(NPD)
        in_maps.append(m)
    return in_maps


def finalize(pmax_list, inputs):
    """Host: combine per-core pooled maxima, apply BN2 + tanh + FC."""
    pm = np.stack([np.asarray(p, np.float32) for p in pmax_list])  # [8,128,256]
    pm = pm.reshape(NCORES, 128, NPAIR * 2, B).max(axis=(0, 2))    # [128, B]
    g2 = np.asarray(inputs['bn2_gamma'], np.float32)
    be2 = np.asarray(inputs['bn2_beta'], np.float32)
    m2 = np.asarray(inputs['bn2_mean'], np.float32)
    v2 = np.asarray(inputs['bn2_var'], np.float32)
    fc_w = np.asarray(inputs['fc_w'], np.float32)
    fc_b = np.asarray(inputs['fc_b'], np.float32)
    s2 = g2 / np.sqrt(v2 + EPS)
    th = np.tanh(pm * s2[:, None] + (be2 - m2 * s2)[:, None])
    return (th.T @ fc_w.T + fc_b).astype(np.float32)


_NC_CACHE = {}


def _get_nc():
    if 'nc' not in _NC_CACHE:
        _NC_CACHE['nc'] = build_bass()
    return _NC_CACHE['nc']


def kernel(**inputs):
    nc = _get_nc()
    in_maps = prep_core_inputs(inputs)
    res = run_bass_kernel_spmd(nc, in_maps, list(range(NCORES)))
    pmax_list = [res.results[i]["pmax"] for i in range(NCORES)]
    return finalize(pmax_list, inputs)
